# revision 1
# baseline (speedup 1.0000x reference)
"""Causal single-head attention (B=4, T=2048, D=1024) on 8 NeuronCores.

Sharding: 2 cores per batch element. Within a batch, core r (r in {0,1})
handles the strided query rows q_glob = 2*j + r (j = 0..1023). The strided
split makes the causal block structure identical on every core (SPMD-safe)
and balances causal work exactly.

Per-core device program (fp32 PSUM accumulation throughout):
  1. K^T = Wk x^T   fp8e4m3 + DoubleRow (256-deep virtual array, 2x rate);
     Wk is pre-scaled x32 into fp8's normal range
  2. V   = x Wv^T   bf16 (fp8 V quantization passes straight to the output:
     measured 3.6e-2 rel vs 1.2e-2 with QK-only fp8)
  3. Q^T = Wq xq^T  fp8e4m3 + DoubleRow, Wq pre-scaled x32
  4. attention per (query block, 128-key tile), S^T formulation: stationary
     K-tile x moving Q gives S^T[k, q] in PSUM (no separate transpose);
     additive causal mask on the two diagonal tiles; exp on ACT (scale
     1/(32*1024) absorbs the two x32 weight scales) writes E bf16 to SBUF;
     E is then itself the stationary operand of out2 += E^T V, and the
     softmax denominator accumulates via an extra 1-column matmul of E
     against a ones vector (numerator and denominator both sum bf16 E,
     consistently). Final per-row divide by the denominator on DVE.
     K^T/Q^T PSUM->SBUF copies run on ACT (Copy activation), V copies on
     DVE, spreading the copy load across queues.

Host side transposes/casts inputs (x twice: fp8 for QK, bf16 for V) and
de-interleaves outputs.
"""
import orjson
import numpy as np
import ml_dtypes

import concourse.bass as bass
import concourse.mybir as mybir
import concourse.tile as tile
from concourse import bass_utils

B, T, D = 4, 2048, 1024
NCORES = 8
P = 128
JQ = T // 2            # local queries per core (1024)
N_QB = JQ // P         # 8 query blocks of 128
N_IT = D // P          # 8 contraction tiles
N_KT = T // P          # 16 key tiles of 128
KC = 512               # key chunk for S matmuls
N_KC = T // KC         # 4
F32 = mybir.dt.float32
BF16 = mybir.dt.bfloat16
SCALE = 1.0 / 32.0     # 1/sqrt(D)
MASK_NEG = -1.0e9

def _split_waits(blocks):
    """The walrus build in this container accepts at most ONE sync-wait per
    instruction; Tile freely emits several. Split extras onto same-engine
    NoOps inserted immediately before the instruction (engine-serial order
    preserves semantics)."""
    n_split = 0
    for blk in blocks:
        insts = blk.get("instructions", [])
        out = []
        for ins in insts:
            si = ins.get("sync_info")
            waits = (si or {}).get("on_wait") or []
            if len(waits) > 1:
                for i, w in enumerate(waits[:-1]):
                    nop = {
                        "engine": ins["engine"],
                        "ins": [],
                        "name": f"{ins['name']}-w{i}",
                        "opcode": "NoOp",
                        "outs": [],
                        "sync_info": {"on_wait": [w], "on_update": []},
                    }
                    if "debug" in ins:
                        nop["debug"] = ins["debug"]
                    out.append(nop)
                    n_split += 1
                si["on_wait"] = [waits[-1]]
            out.append(ins)
        blk["instructions"] = out
        _split_waits(blk.get("blocks", []) or [])
    return n_split


class _Bass(bass.Bass):
    def to_json_bytes(self):
        d = orjson.loads(super().to_json_bytes())
        for f in d["functions"]:
            _split_waits(f["blocks"])
        return orjson.dumps(d)


def n_kc_of(qb: int) -> int:
    # chunks of 512 keys needed by query block qb (covers q_glob < 256*(qb+1))
    return qb // 2 + 1


# Pair-split K/V projection + AllGather exchange trades ~55us of PE work for
# a ~100us (measured) pair-gather of 4 MiB: roughly perf-neutral on HW and
# worse on the cost model, so it stays off. The code path is kept for tuning.
SPLIT_KV = False
# fp8e4m3 + DoubleRow perf mode on the Q/K projections only: 2 contraction
# subtiles per matmul (virtual 256-deep array), ~1.4x PE throughput on those
# phases. Wq/Wk are pre-scaled x32 into fp8's normal range; the 32*32 factor
# cancels inside the softmax scale. V stays bf16 end-to-end: fp8 V errors
# pass straight to the output (measured 3.6e-2 rel), QK-only is 8e-3.
FP8_PROJ = True
F8 = mybir.dt.float8e4
W_SCALE = 32.0


def build_nc(split_kv: bool = SPLIT_KV, fp8_proj: bool = FP8_PROJ) -> bass.Bass:
    nc = _Bass("TRN2", debug=False, num_devices=NCORES)

    QK_DT = F8 if fp8_proj else BF16
    KSTEP = 2 if fp8_proj else 1
    PMODE = mybir.MatmulPerfMode.DoubleRow if fp8_proj else None
    scale_eff = SCALE / (W_SCALE * W_SCALE) if fp8_proj else SCALE

    TH = T // 2 if split_kv else T
    # split_kv: core r of a pair owns keys [r*1024, r*1024+1024); K/V are
    # computed for the half then pair-AllGathered. Otherwise each core
    # computes K/V for all keys from the full x^T.
    xhT = nc.dram_tensor("xhT", [D, TH], QK_DT, kind="ExternalInput")
    xqT = nc.dram_tensor("xqT", [D, JQ], QK_DT, kind="ExternalInput")
    wqT = nc.dram_tensor("wqT", [D, D], QK_DT, kind="ExternalInput")
    wkT = nc.dram_tensor("wkT", [D, D], QK_DT, kind="ExternalInput")
    wvT = nc.dram_tensor("wvT", [D, D], BF16, kind="ExternalInput")
    if fp8_proj:
        # bf16 copy of the local keys' x^T for the V projection
        xhTb = nc.dram_tensor("xhTb", [D, TH], BF16, kind="ExternalInput")
    maskadd = nc.dram_tensor("maskadd", [2, P, P], F32, kind="ExternalInput")
    out = nc.dram_tensor("out", [JQ, D], F32, kind="ExternalOutput")

    with tile.TileContext(nc) as tc:
        with (
            tc.tile_pool(name="big", bufs=1) as big,
            tc.tile_pool(name="wpool", bufs=1) as wpool,
            tc.tile_pool(name="small", bufs=2) as small,
            tc.tile_pool(name="pwork", bufs=4) as pwork,
            tc.tile_pool(name="dram", bufs=1, space="DRAM") as dram,
            tc.tile_pool(name="mm", bufs=3, space="PSUM") as mm,
            tc.tile_pool(name="denp", bufs=1, space="PSUM") as denp,
            tc.tile_pool(name="o2p", bufs=2, space="PSUM") as o2p,
        ):

            def load_w(dram, dt, tag, chunk=D):
                w = wpool.tile([P, N_IT, D], dt, tag=tag)
                w_r = dram.rearrange("(it p) o -> it p o", p=P)
                for oc0 in range(0, D, chunk):
                    for it in range(N_IT):
                        nc.sync.dma_start(
                            w[:, it, oc0:oc0 + chunk], w_r[it][:, oc0:oc0 + chunk]
                        )
                return w

            # wk/xh interleaved per contraction tile so the opening K-proj
            # accumulation group becomes runnable as soon as possible
            wk = wpool.tile([P, N_IT, D], QK_DT, tag="wk")
            wk_r = wkT.rearrange("(it p) o -> it p o", p=P)
            xh = big.tile([P, N_IT, TH], QK_DT, tag="xh")
            xhT_r = xhT.rearrange("(it p) t -> it p t", p=P)
            for it in range(N_IT):
                nc.sync.dma_start(wk[:, it, :], wk_r[it])
                # tc0 rides the otherwise-idle ACT hwdge queue: wk and the
                # opening xh chunk stream in parallel at kernel start
                nc.scalar.dma_start(xh[:, it, 0:KC], xhT_r[it][:, 0:KC])
            for tc4 in range(1, TH // KC):
                for it in range(N_IT):
                    nc.sync.dma_start(
                        xh[:, it, tc4 * KC:(tc4 + 1) * KC],
                        xhT_r[it][:, tc4 * KC:(tc4 + 1) * KC],
                    )


            # ---- resident K^T / V / Q^T for the attention phase ----
            kt_sb = big.tile([P, N_IT, T], BF16, tag="kt")
            v_sb = big.tile([P, N_KT, D], BF16, tag="v")
            qt_sb = big.tile([P, N_IT, JQ], BF16, tag="qt")

            if split_kv:
                # DRAM bounce buffers for the pair AllGather of local K^T / V
                inb = dram.tile([2, TH, D], BF16, tag="inb", name="inb")
                outb = dram.tile([2, 2, TH, D], BF16, tag="outb", name="outb")

            # ---- K^T_local[o, t] = sum_i WkT[i,o] * xhT[i,t] ----
            for tc4 in range(TH // KC):
                for ot in range(N_IT):
                    acc = mm.tile([P, KC], F32, tag="mm512")
                    for it in range(0, N_IT, KSTEP):
                        nc.tensor.matmul(
                            acc[:],
                            wk[:, it:it + KSTEP, ot * P:(ot + 1) * P],
                            xh[:, it:it + KSTEP, tc4 * KC:(tc4 + 1) * KC],
                            start=(it == 0), stop=(it == N_IT - KSTEP),
                            perf_mode=PMODE,
                        )
                    if split_kv:
                        stg = pwork.tile([P, KC], BF16, tag="kvst", name="stg")
                        nc.vector.tensor_copy(stg[:], acc[:])
                        nc.sync.dma_start(
                            inb[0, ot * P:(ot + 1) * P, tc4 * KC:(tc4 + 1) * KC], stg[:]
                        )
                    else:
                        nc.scalar.activation(
                            kt_sb[:, ot, tc4 * KC:(tc4 + 1) * KC], acc[:],
                            mybir.ActivationFunctionType.Copy,
                        )

            # ---- V_local[t, o] = sum_i xhT[i,t] * WvT[i,o] ----  (bf16)
            wv = wpool.tile([P, N_IT, D], BF16, tag="wv")
            wv_r = wvT.rearrange("(it p) o -> it p o", p=P)
            if fp8_proj:
                xhb = big.tile([P, N_IT, TH], BF16, tag="xhb")
                xhTb_r = xhTb.rearrange("(it p) t -> it p t", p=P)
            else:
                xhb = xh
            for it in range(N_IT):
                nc.sync.dma_start(wv[:, it, :], wv_r[it])
            if fp8_proj:
                for tc4 in range(TH // KC):
                    for it in range(N_IT):
                        nc.sync.dma_start(
                            xhb[:, it, tc4 * KC:(tc4 + 1) * KC],
                            xhTb_r[it][:, tc4 * KC:(tc4 + 1) * KC],
                        )
            xq = big.tile([P, N_IT, JQ], QK_DT, tag="xq")
            xqT_r = xqT.rearrange("(it p) t -> it p t", p=P)
            for it in range(N_IT):
                nc.sync.dma_start(xq[:, it, :], xqT_r[it])
            for tt in range(TH // P):
                for oc in range(D // KC):
                    acc = mm.tile([P, KC], F32, tag="mm512")
                    for it in range(N_IT):
                        nc.tensor.matmul(
                            acc[:],
                            xhb[:, it, tt * P:(tt + 1) * P],
                            wv[:, it, oc * KC:(oc + 1) * KC],
                            start=(it == 0), stop=(it == N_IT - 1),
                        )
                    if split_kv:
                        stg = pwork.tile([P, KC], BF16, tag="kvst", name="stg")
                        nc.vector.tensor_copy(stg[:], acc[:])
                        nc.sync.dma_start(
                            inb[1, tt * P:(tt + 1) * P, oc * KC:(oc + 1) * KC], stg[:]
                        )
                    else:
                        nc.vector.tensor_copy(
                            v_sb[:, tt, oc * KC:(oc + 1) * KC], acc[:]
                        )

            if split_kv:
                # ---- pair AllGather: both halves of K^T and V ----
                nc.gpsimd.collective_compute(
                    "AllGather",
                    mybir.AluOpType.bypass,
                    replica_groups=[[0, 1], [2, 3], [4, 5], [6, 7]],
                    ins=[inb.opt()],
                    outs=[outb.opt()],
                )
                for h in range(2):
                    for ot in range(N_IT):
                        nc.sync.dma_start(
                            kt_sb[:, ot, h * TH:(h + 1) * TH],
                            outb[h, 0, ot * P:(ot + 1) * P, :],
                        )
                for tt16 in range(N_KT):
                    h, tl = tt16 // (TH // P), tt16 % (TH // P)
                    nc.sync.dma_start(
                        v_sb[:, tt16, :],
                        outb[h, 1, tl * P:(tl + 1) * P, :],
                    )

            # ---- Q^T projection: Q^T[o, j] = sum_i WqT[i,o] * xqT[i,j] ----
            wq = load_w(wqT, QK_DT, "wq")
            for ot in range(N_IT):
                for jc in range(JQ // KC):
                    acc = mm.tile([P, KC], F32, tag="mm512")
                    for it in range(0, N_IT, KSTEP):
                        nc.tensor.matmul(
                            acc[:],
                            wq[:, it:it + KSTEP, ot * P:(ot + 1) * P],
                            xq[:, it:it + KSTEP, jc * KC:(jc + 1) * KC],
                            start=(it == 0), stop=(it == N_IT - KSTEP),
                            perf_mode=PMODE,
                        )
                    nc.scalar.activation(
                        qt_sb[:, ot, jc * KC:(jc + 1) * KC], acc[:],
                        mybir.ActivationFunctionType.Copy,
                    )

            # ---- attention (S^T formulation), pipelined over (qb, kt) ----
            # S^T[k, q] per 128-key tile via stationary K-tiles / moving Q;
            # the exp'd tile E then serves directly as the stationary operand
            # of the PV matmuls, so P is never transposed (no PE-transpose,
            # no DVE copy). The softmax denominator accumulates via an extra
            # 1-column matmul against a ones vector on the same stationary.
            masksT = big.tile([P, 2, P], F32, tag="masks")
            # DRAM [2,128,128] -> partition-major per mask
            nc.sync.dma_start(masksT[:], maskadd.rearrange("m p f -> p m f"))
            ones_sb = big.tile([P, 1], BF16, tag="ones")
            nc.any.memset(ones_sb[:], 1.0)

            tiles = [(qb, kt) for qb in range(N_QB) for kt in range(2 * qb + 2)]
            state = {}  # per live qb: out2 + denominator PSUM

            def emit_st_phase(qb, kt):
                if kt == 0:
                    state[qb] = {
                        "out2": o2p.tile([P, D], F32, tag="out2", name="out2"),
                        "den": denp.tile([P, 1], F32, tag="den", name="den"),
                    }
                st = mm.tile([P, P], F32, tag="mm512")
                for ot in range(N_IT):
                    nc.tensor.matmul(
                        st[:],
                        kt_sb[:, ot, kt * P:(kt + 1) * P],
                        qt_sb[:, ot, qb * P:(qb + 1) * P],
                        start=(ot == 0), stop=(ot == N_IT - 1),
                    )
                m = kt - 2 * qb
                if m >= 0:  # one of the two diagonal tiles: additive mask
                    nc.vector.tensor_add(st[:], st[:], masksT[:, m, :])
                e = pwork.tile([P, P], BF16, tag="e")
                nc.scalar.activation(
                    e[:], st[:], mybir.ActivationFunctionType.Exp, scale=scale_eff
                )
                return e

            def emit_pv_phase(qb, kt, e):
                last = kt == 2 * qb + 1
                out2 = state[qb]["out2"]
                den = state[qb]["den"]
                for oc in range(D // KC):
                    nc.tensor.matmul(
                        out2[:, oc * KC:(oc + 1) * KC],
                        e[:], v_sb[:, kt, oc * KC:(oc + 1) * KC],
                        start=(kt == 0), stop=last,
                    )
                nc.tensor.matmul(den[:], e[:], ones_sb[:], start=(kt == 0), stop=last)
                if last:
                    linv = small.tile([P, 1], F32, tag="linv")
                    nc.vector.reciprocal(linv[:], den[:])
                    for oc in range(D // KC):
                        oh = small.tile([P, KC], F32, tag="oh")
                        nc.vector.tensor_scalar_mul(
                            oh[:], out2[:, oc * KC:(oc + 1) * KC], linv[:]
                        )
                        nc.sync.dma_start(
                            out[qb * P:(qb + 1) * P, oc * KC:(oc + 1) * KC], oh[:]
                        )
                    del state[qb]

            prev = None
            for qb, kt in tiles:
                e = emit_st_phase(qb, kt)
                if prev is not None:
                    emit_pv_phase(*prev)
                prev = (qb, kt, e)
            emit_pv_phase(*prev)

    return nc


_NC = {}


def _get_nc(split_kv: bool = SPLIT_KV, fp8_proj: bool = FP8_PROJ):
    key = (split_kv, fp8_proj)
    if key not in _NC:
        _NC[key] = build_nc(split_kv, fp8_proj)
    return _NC[key]


def _prep_in_maps(inputs, Wq, Wk, Wv, split_kv: bool = SPLIT_KV,
                  fp8_proj: bool = FP8_PROJ):
    inputs = np.asarray(inputs, dtype=np.float32)
    Wq = np.asarray(Wq, dtype=np.float32)
    Wk = np.asarray(Wk, dtype=np.float32)
    Wv = np.asarray(Wv, dtype=np.float32)

    bf = ml_dtypes.bfloat16
    qk = ml_dtypes.float8_e4m3 if fp8_proj else bf
    ws = np.float32(W_SCALE) if fp8_proj else np.float32(1.0)
    wqT = np.ascontiguousarray(Wq.T * ws).astype(qk)
    wkT = np.ascontiguousarray(Wk.T * ws).astype(qk)
    wvT = np.ascontiguousarray(Wv.T).astype(bf)

    in_maps = []
    for c in range(NCORES):
        b, r = c // 2, c % 2
        xb = inputs[b]                                  # [T, D]
        if split_kv:
            xhTf = np.ascontiguousarray(xb[r * (T // 2):(r + 1) * (T // 2), :].T)
        else:
            xhTf = np.ascontiguousarray(xb.T)
        xhT = xhTf.astype(qk)
        xqT = np.ascontiguousarray(xb[r::2, :].T).astype(qk)  # [D, JQ]
        # additive causal mask for the two diagonal 128-key tiles of
        # each query block: S^T layout [key ks, query j], q_glob = 256*qb+2j+r,
        # key = 256*qb + 128*m + ks -> keep iff 128*m + ks <= 2j + r
        ks_idx = np.arange(P)[:, None]
        j_idx = np.arange(P)[None, :]
        masks = np.empty((2, P, P), dtype=np.float32)
        for m in range(2):
            keep = 128 * m + ks_idx <= 2 * j_idx + r
            masks[m] = np.where(keep, 0.0, MASK_NEG)
        im = {
            "xhT": xhT, "xqT": xqT,
            "wqT": wqT, "wkT": wkT, "wvT": wvT,
            "maskadd": masks,
        }
        if fp8_proj:
            im["xhTb"] = xhTf.astype(bf)
        in_maps.append(im)
    return in_maps


def _gather(res):
    result = np.empty((B, T, D), dtype=np.float32)
    for c in range(NCORES):
        b, r = c // 2, c % 2
        result[b, r::2, :] = res.results[c]["out"]
    return result


def kernel(inputs, Wq, Wk, Wv):
    in_maps = _prep_in_maps(inputs, Wq, Wk, Wv)
    nc = _get_nc()
    res = bass_utils.run_bass_kernel_spmd(nc, in_maps, core_ids=list(range(NCORES)))
    return _gather(res)


def run_traced(inputs, Wq, Wk, Wv):
    """Like kernel() but with NTFF tracing; returns BassKernelResults
    (exec_time_ns, trace path). For test.py only."""
    in_maps = _prep_in_maps(inputs, Wq, Wk, Wv)
    nc = _get_nc()
    res = bass_utils.run_bass_kernel_spmd(
        nc, in_maps, core_ids=list(range(NCORES)), trace=True
    )
    res.full_output = _gather(res)
    return res

